# revision 1
# baseline (speedup 1.0000x reference)
"""Trainium2 Bass kernel for DisplaceChannel.

Math (per channel c, group f = c // 16):
  off_px  = offset[f] * 64;  off_int = round(off_px);  sub = off_px - off_int
  shifted[y, x] = x[y - dy, x - dx]  (zero outside), dy/dx = off_int
  out = depthwise 3x3 SAME conv of `shifted` with a normalized Gaussian
        kernel built from `sub`.  The Gaussian kernel is separable:
        kern = u (vertical taps) ⊗ v (horizontal taps).

Implementation: batch-parallel over 8 cores (2 batches per core), all 48
groups on every core so the SPMD program is identical across cores.  The
shift geometry and 1-D taps are computed on host from the actual `offset`
input and baked into the program (access patterns + inline weight tensor).

Channel blocks are consecutive runs of up to 8 groups chosen by a small DP
to minimize the union row-band per block: a group shifted by dy has only
rows [max(0,dy), 64+min(0,dy)) nonzero after the shift, so compute and
DMA are restricted to the block's row band (plus the 1-row conv widening)
and the all-zero output rows are stored from a static zero tile.

Per (batch, block) tile, channels on partitions:
  - one HWDGE DMA per group loading only the valid window of the shifted
    image straight from DRAM into a persistent pre-zeroed S tile
  - horizontal 3-tap pass on DVE into T (per-partition scalar taps)
  - vertical 3-tap pass on DVE into O over the band widened by 1 row
  - DMA O band back to DRAM, plus zero-tile stores for the rows outside
"""

import os
import sys
from contextlib import ExitStack

import numpy as np

for _p in ("/opt/trn_rl_repo", "/root/.axon_site/_ro/trn_rl_repo"):
    if os.path.isdir(_p) and _p not in sys.path:
        sys.path.append(_p)

import concourse.bass as bass
import concourse.bacc as bacc
import concourse.mybir as mybir
import concourse.tile as tile
from concourse.bass_utils import run_bass_kernel_spmd

H = W = 64
C = 768
B = 16
N_CORES = 8
BPC = B // N_CORES          # batches per core
P = 128                     # partitions
NGRP = 48
GSZ = 16                    # channels per group
SCALE = 64.0
SIGMA = 0.5
FP32 = mybir.dt.float32
MULT = mybir.AluOpType.mult
ADD = mybir.AluOpType.add


def _geometry(offset: np.ndarray):
    """Integer shifts and separable 1-D taps per group, matching reference."""
    off_px = offset.astype(np.float32) * np.float32(SCALE)
    off_int = np.round(off_px)
    sub = off_px - off_int                      # [48, 2] (x, y)
    dx = off_int[:, 0].astype(np.int64)
    dy = off_int[:, 1].astype(np.int64)
    r = (np.arange(3, dtype=np.float32) - 1.0).astype(np.float32)
    ex = np.exp(-((r[None, :] + sub[:, 0:1]) ** 2) / (2.0 * SIGMA * SIGMA))
    ey = np.exp(-((r[None, :] + sub[:, 1:2]) ** 2) / (2.0 * SIGMA * SIGMA))
    v = ex / ex.sum(1, keepdims=True)           # [48, 3] horizontal taps
    u = ey / ey.sum(1, keepdims=True)           # [48, 3] vertical taps
    return dx, dy, v.astype(np.float32), u.astype(np.float32)


def _row_window(dyg: int):
    """Nonzero row range [r0, r1) of the shifted image for shift dy."""
    r0 = max(0, dyg)
    r1 = H + min(0, dyg)
    return r0, max(r0, r1)


def _partition_blocks(dy):
    """Split groups 0..47 into consecutive runs of <=8 groups minimizing the
    summed union row-band height (DVE work is proportional to it)."""
    r0s = [_row_window(int(d))[0] for d in dy]
    r1s = [_row_window(int(d))[1] for d in dy]
    INF = float("inf")
    best = [INF] * (NGRP + 1)
    prev = [0] * (NGRP + 1)
    best[0] = 0.0
    for e in range(1, NGRP + 1):
        for s in range(max(0, e - 8), e):
            band = max(r1s[s:e]) - min(r0s[s:e])
            cost = best[s] + band * 352.0 + 2500.0
            if cost < best[e]:
                best[e] = cost
                prev[e] = s
    cuts = []
    e = NGRP
    while e > 0:
        s = prev[e]
        cuts.append((s, e))
        e = s
    blocks = []
    for s, e in reversed(cuts):
        r0 = min(r0s[s:e])
        r1 = max(r1s[s:e])
        blocks.append((s, e, r0, r1))
    return blocks


def _build(offset: np.ndarray) -> bass.Bass:
    dx, dy, v, u = _geometry(offset)
    blocks = _partition_blocks(dy)

    # Per-block per-partition tap table: wnp[blk, c_local, j], j = v0 v1 v2 u0 u1 u2
    nblk = len(blocks)
    wnp = np.zeros((nblk, P, 6), dtype=np.float32)
    for bi, (s, e, _, _) in enumerate(blocks):
        for gl, g in enumerate(range(s, e)):
            sl = slice(gl * GSZ, (gl + 1) * GSZ)
            wnp[bi, sl, 0:3] = v[g]
            wnp[bi, sl, 3:6] = u[g]

    nc = bacc.Bacc("TRN2", target_bir_lowering=False, debug=False)
    x_in = nc.dram_tensor("x", [BPC, C, H, W], FP32, kind="ExternalInput")
    y_out = nc.dram_tensor("y", [BPC, C, H, W], FP32, kind="ExternalOutput")
    w_dram = nc.inline_tensor(wnp, name="taps")

    with tile.TileContext(nc) as tc, ExitStack() as ctx:
        w_pool = ctx.enter_context(tc.tile_pool(name="w", bufs=1))
        s_pool = ctx.enter_context(tc.tile_pool(name="s", bufs=1))
        t_pool = ctx.enter_context(tc.tile_pool(name="t", bufs=4))
        o_pool = ctx.enter_context(tc.tile_pool(name="o", bufs=3))

        wt = []
        for bi in range(nblk):
            wtile = w_pool.tile([P, 6], FP32, name=f"w{bi}", tag=f"w{bi}")
            nc.gpsimd.dma_start(wtile[:], w_dram[bi])
            wt.append(wtile)

        # Persistent S tile per block (band rows only), zeroed once.  Every
        # reuse DMAs the exact same per-group windows, so the zero complement
        # stays valid without per-iteration memsets.
        s_tiles = []
        for bi, (s, e, r0, r1) in enumerate(blocks):
            S = s_pool.tile([P, r1 - r0, W], FP32, name=f"S{bi}", tag=f"S{bi}")
            s_tiles.append(S)
        # zero in processing order (largest band first) so the first tiles'
        # loads are unblocked earliest
        for bi in sorted(range(nblk), key=lambda i: -(blocks[i][3] - blocks[i][2])):
            nc.gpsimd.memset(s_tiles[bi][:], 0.0)

        # Per-tile emission stages, software-pipelined one tile deep so the
        # cross-engine center-tap dependencies (ACT <-> DVE) never stall an
        # engine stream: while DVE runs tile j's horizontal taps, ACT
        # finishes tile j-1's vertical center.
        def emit_load_and_h(b, bi):
            s, e, r0, r1 = blocks[bi]
            S = s_tiles[bi]
            h = r1 - r0
            for gl, g in enumerate(range(s, e)):
                dyg, dxg = int(dy[g]), int(dx[g])
                gr0, gr1 = _row_window(dyg)
                ny, nx = gr1 - gr0, W - abs(dxg)
                if ny <= 0 or nx <= 0:
                    continue
                ys = max(0, -dyg)
                xs, xd = max(0, -dxg), max(0, dxg)
                ch0 = s * GSZ + gl * GSZ
                ldeng = nc.sync if gl % 2 == 0 else nc.scalar
                ldeng.dma_start(
                    S[gl * GSZ:(gl + 1) * GSZ,
                      gr0 - r0:gr0 - r0 + ny, xd:xd + nx],
                    x_in[b, ch0:ch0 + GSZ, ys:ys + ny, xs:xs + nx],
                )

            wv0, wv1, wv2 = (wt[bi][:, j:j + 1] for j in range(3))

            # horizontal pass into T: center tap on ACT, +-1 taps on DVE
            # (full 128 partitions; unused partitions have zero taps and
            # zero S, so T is fully defined).  T covers global rows
            # [r0-2, r1+2) as local 0..h+4; border rows 0,1 and h+2,h+3
            # kept zero so vertical taps can read one row beyond.
            T = t_pool.tile([P, H + 4, W], FP32, name="T", tag="T")
            nc.gpsimd.memset(T[:, 0:h + 4:h + 2, :], 0.0)
            nc.gpsimd.memset(T[:, 1:h + 4:h + 2, :], 0.0)
            nc.scalar.mul(T[:, 2:h + 2, :], S[:, :, :], wv1)
            nc.vector.scalar_tensor_tensor(
                T[:, 2:h + 2, 1:W], S[:, :, 0:W - 1], wv0,
                T[:, 2:h + 2, 1:W], MULT, ADD)
            nc.vector.scalar_tensor_tensor(
                T[:, 2:h + 2, 0:W - 1], S[:, :, 1:W], wv2,
                T[:, 2:h + 2, 0:W - 1], MULT, ADD)
            return T

        def emit_v_and_store(b, bi, T):
            s, e, r0, r1 = blocks[bi]
            np_used = (e - s) * GSZ
            v0 = max(r0 - 1, 0)
            v1 = min(r1 + 1, H)
            nv = v1 - v0
            wu0, wu1, wu2 = (wt[bi][:, j:j + 1] for j in range(3, 6))

            # vertical pass into O over global rows [v0, v1); rows outside
            # the band are zeroed on-chip so the store is one full-height
            # contiguous DMA per channel.
            O = o_pool.tile([P, H, W], FP32, name="O", tag="O")
            if v0 > 0:
                nc.gpsimd.memset(O[:, 0:v0, :], 0.0)
            if v1 < H:
                nc.gpsimd.memset(O[:, v1:H, :], 0.0)
            nc.scalar.mul(
                O[:, v0:v1, :], T[:, v0 - r0 + 2:v1 - r0 + 2, :], wu1)
            nc.vector.scalar_tensor_tensor(
                O[:, v0:v1, :], T[:, v0 - r0 + 1:v1 - r0 + 1, :], wu0,
                O[:, v0:v1, :], MULT, ADD)
            nc.vector.scalar_tensor_tensor(
                O[:, v0:v1, :], T[:, v0 - r0 + 3:v1 - r0 + 3, :], wu2,
                O[:, v0:v1, :], MULT, ADD)

            ch0, ch1 = s * GSZ, e * GSZ
            nc.gpsimd.dma_start(y_out[b, ch0:ch1, :, :], O[:np_used, :, :])

        order = sorted(range(nblk), key=lambda i: -(blocks[i][3] - blocks[i][2]))
        tiles = [(b, bi) for b in range(BPC) for bi in order]
        from collections import deque
        pend = deque()                   # (b, bi, T) awaiting vertical pass
        DEPTH = 2
        for b, bi in tiles:
            T = emit_load_and_h(b, bi)
            pend.append((b, bi, T))
            if len(pend) > DEPTH:
                emit_v_and_store(*pend.popleft())
        while pend:
            emit_v_and_store(*pend.popleft())

    nc.compile()
    return nc


def _run(x: np.ndarray, offset: np.ndarray, trace: bool = False):
    x = np.ascontiguousarray(x, dtype=np.float32)
    offset = np.ascontiguousarray(offset, dtype=np.float32)
    nc = _build(offset)
    in_maps = [
        {"x": x[k * BPC:(k + 1) * BPC]} for k in range(N_CORES)
    ]
    res = run_bass_kernel_spmd(
        nc, in_maps, core_ids=list(range(N_CORES)), trace=trace
    )
    out = np.concatenate([res.results[k]["y"] for k in range(N_CORES)], axis=0)
    return out.astype(np.float32), res


def kernel(x: np.ndarray, offset: np.ndarray) -> np.ndarray:
    return _run(x, offset)[0]

